# revision 29
# baseline (speedup 1.0000x reference)
"""Single-head attention with interleaved RoPE — Trainium2 Bass kernel (v4).

Problem: B=32, S=1024, D=768 fp32.
  Q = x@Wq.T + bq; K = x@Wk.T + bk; V = x@Wv.T + bv
  Q,K := interleaved RoPE over sequence positions
  out = softmax(Q K^T / sqrt(D)) @ V

v8 (vs v3 at ~383us; the fp16 PE-streaming roofline for this layout is
348.5us and the steady state already runs exactly at roofline +2.8ns/MM
NX dispatch, so all recoverable slack is startup, batch-0, and tail):
  - fp8 was evaluated and rejected: DoubleRow doubles PE rate but exact
    numpy emulation of e4m3 quantization busts the 2e-2 gate 2.4-3.6x
    for EVERY stage (scores 5.7e-2, AV 4.8e-2), and error-compensated
    splits cost back exactly the 2x.
  - Host layouts make every batch-0 critical DMA a large-packet lump:
    wq/wk c-block-major [P,3,EO,2,128], x j-block-major [P,4,EO,256],
    trig [P,2(h),2(cos/sin),NPAIR,512], biases packed into one [P,784]
    tensor (separate bq/bk loads were 128x 24B packets, poisoning the
    SDMA round-robin).
  - The 16 SDMA engines round-robin across ACTIVE queues per packet,
    so queue order IS priority: both HW-DGE queues (sync, scalar) are
    loaded strictly by compute deadline, the first matmul group's
    operands split across both queues in parallel.  wv rides the
    gpsimd SWDGE gated behind a RoPE output read (a bare dma_start
    gets scheduler-hoisted to t=0 and steals early bandwidth).
  - 26 PE warm-up dummies (5.5us busy) guarantee the free-running
    3.4us HAM window sees a full busy window, so the clock-gate lifts
    to 2.4GHz before the first real matmuls (16 raced it and losing
    runs ran all of batch 0 at 1.2GHz).
  - RoPE temps in fp16 (GPSIMD combines get the 2x_1P packed mode);
    V-bias drains on gpsimd; K-projection before Q in the second
    half so the scores matmuls never wait on the last RoPE chain.
  - Tail: the final q-tiles store each 512/256-col half as soon as it
    is scaled so the last transfer overlaps the last activation.
"""

import numpy as np

import concourse.bass as bass
import concourse.mybir as mybir
import concourse.tile as tile
from concourse import bacc
from concourse.bass_utils import run_bass_kernel_spmd

F32 = mybir.dt.float32
F16 = mybir.dt.float16

B, S, D = 32, 1024, 768
NCORES = 8
BPC = B // NCORES          # batches per core
P = 128
EO = D // P                # 6 feature chunks
KO = S // P                # 8 sequence tiles
JB = S // 256              # 4 j-blocks of 256 seq positions
HALF = D // 2              # 384
NPAIR = HALF // P          # 3 rope chunk pairs
DA = D + 2                 # V augmented: ones column (row sums) + zero pad
ROPE_BASE = 10000.0
INV_SQRT_D = float(1.0 / np.sqrt(np.float32(D)))
N_WARMUP_MM = 26           # PE pre-warm dummy matmuls (N=256 each).
# 26 x 213ns = 5.5us of sustained PE-busy: the free-running 3.4us HAM
# window is guaranteed to see one full busy window, so the clock-gate
# lifts to 2.4GHz before the first real (DMA-gated) matmuls run.  16
# was exactly one window and raced it: runs that lost started the
# whole first batch at 1.2GHz.

_CACHE: dict = {}


def _build_nc():
    nc = bacc.Bacc("TRN2", target_bir_lowering=False, debug=False,
                   num_devices=NCORES)

    xt_d = nc.dram_tensor("xt", (BPC, P, JB, EO, 256), F16,
                          kind="ExternalInput").ap()
    wq_d = nc.dram_tensor("wq", (P, NPAIR, EO, 2, P), F16,
                          kind="ExternalInput").ap()
    wk_d = nc.dram_tensor("wk", (P, NPAIR, EO, 2, P), F16,
                          kind="ExternalInput").ap()
    wv_d = nc.dram_tensor("wv", (P, EO, DA), F16, kind="ExternalInput").ap()
    # bq | bk | bvb packed into one transfer: separate loads would each
    # be 128 tiny 24B packets, poisoning the SDMA round-robin.  fp16:
    # biases are ~0.01 magnitude, quantization ~5e-6 absolute.
    bias_d = nc.dram_tensor("bias", (P, 2 * EO + DA), F16,
                            kind="ExternalInput").ap()
    trig_d = nc.dram_tensor("trig", (P, 2, 2, NPAIR, 512), F16,
                            kind="ExternalInput").ap()
    out_d = nc.dram_tensor("out", (BPC, S, D), F32, kind="ExternalOutput").ap()

    with tile.TileContext(nc) as tc:
        _body(tc, xt_d, wq_d, wk_d, wv_d, bias_d, trig_d, out_d)
    nc.compile()
    return nc


def _body(tc, xt_d, wq_d, wk_d, wv_d, bias_d, trig_d, out_d):
    nc = tc.nc
    Add = mybir.AluOpType.add
    Sub = mybir.AluOpType.subtract
    Mult = mybir.AluOpType.mult
    Exp = mybir.ActivationFunctionType.Exp
    Ident = mybir.ActivationFunctionType.Identity

    with (
        tc.tile_pool(name="consts", bufs=1) as consts,
        tc.tile_pool(name="xt", bufs=2) as xt_pool,
        tc.tile_pool(name="qkv", bufs=1) as qkv,
        tc.tile_pool(name="rope_tmp", bufs=8) as rope_tmp,
        tc.tile_pool(name="osb", bufs=2) as osb_pool,
        tc.tile_pool(name="small", bufs=4) as small,
        tc.tile_pool(name="ps", bufs=4, space="PSUM") as ps,
    ):
        bias_s = consts.tile([P, 2 * EO + DA], F16, tag="bias")
        bq_s = bias_s[:, 0:EO]
        bk_s = bias_s[:, EO:2 * EO]
        bvb_s = bias_s[:, 2 * EO:2 * EO + DA]
        trig_s = consts.tile([P, 2, 2, NPAIR, 512], F16, tag="trig")
        wq_s = consts.tile([P, NPAIR, EO, 2, P], F16, tag="wq")
        wk_s = consts.tile([P, NPAIR, EO, 2, P], F16, tag="wk")
        wv_s = consts.tile([P, EO, DA], F16, tag="wv")

        # warm-up operand for the PE pre-warm matmuls; memset on the
        # vector engine, which clears the framework prologue early and
        # has no other work until the first RoPE group
        warm_s = consts.tile([P, 256], F16, tag="warm")
        nc.vector.memset(warm_s[:], 0.0)

        # batch-0 critical-path loads.  sync and scalar are the two
        # HW-DGE queues; the 16 SDMA engines round-robin across active
        # queues per packet, so order IS priority.  First real matmul
        # group needs wq-c0 (sync lump 1) + xt-j0 (scalar lump 1).
        # batch-0 loads, strictly ordered by compute deadline on each of
        # the two HW-DGE queues (the 16 SDMA engines round-robin across
        # active queues per packet; aggregate ~0.36 MB/us with both
        # active).  The first matmul group's operands (wq-c0, xt-j0) are
        # split across BOTH queues so each rides half the bandwidth in
        # parallel.  DVE-side inputs (trig, bias) may land ~2us past
        # their first use — the PSUM slot rotation absorbs that; PE-side
        # inputs (wq/wk/xt lumps) must not be late.
        xt0_t = xt_pool.tile([P, JB, EO, 256], F16, tag="xt")
        nc.sync.dma_start(wq_s[:, 0, 0:3], wq_d[:, 0, 0:3])
        nc.scalar.dma_start(wq_s[:, 0, 3:6], wq_d[:, 0, 3:6])
        nc.sync.dma_start(xt0_t[:, 0, 0:3], xt_d[0, :, 0, 0:3])
        nc.scalar.dma_start(xt0_t[:, 0, 3:6], xt_d[0, :, 0, 3:6])
        nc.sync.dma_start(wq_s[:, 1], wq_d[:, 1])
        nc.scalar.dma_start(xt0_t[:, 1], xt_d[0, :, 1])
        nc.sync.dma_start(trig_s[:, 0, 0, 0], trig_d[:, 0, 0, 0])
        nc.scalar.dma_start(bias_s[:], bias_d[:])
        nc.sync.dma_start(wq_s[:, 2], wq_d[:, 2])
        nc.scalar.dma_start(trig_s[:, 0, 1, 0], trig_d[:, 0, 1, 0])
        nc.sync.dma_start(wk_s[:, 0], wk_d[:, 0])
        nc.scalar.dma_start(xt0_t[:, 2], xt_d[0, :, 2])
        nc.sync.dma_start(trig_s[:, 0, 0, 1], trig_d[:, 0, 0, 1])
        nc.scalar.dma_start(trig_s[:, 0, 1, 1], trig_d[:, 0, 1, 1])
        nc.sync.dma_start(trig_s[:, 0, 0, 2], trig_d[:, 0, 0, 2])
        nc.scalar.dma_start(trig_s[:, 0, 1, 2], trig_d[:, 0, 1, 2])
        nc.scalar.dma_start(xt0_t[:, 3], xt_d[0, :, 3])
        nc.sync.dma_start(trig_s[:, 1, 0, 0], trig_d[:, 1, 0, 0])
        nc.scalar.dma_start(trig_s[:, 1, 1, 0], trig_d[:, 1, 1, 0])
        nc.sync.dma_start(trig_s[:, 1, 0, 1], trig_d[:, 1, 0, 1])
        nc.scalar.dma_start(trig_s[:, 1, 1, 1], trig_d[:, 1, 1, 1])
        nc.sync.dma_start(trig_s[:, 1, 0, 2], trig_d[:, 1, 0, 2])
        nc.scalar.dma_start(trig_s[:, 1, 1, 2], trig_d[:, 1, 1, 2])
        nc.sync.dma_start(wk_s[:, 1], wk_d[:, 1])
        nc.sync.dma_start(wk_s[:, 2], wk_d[:, 2])

        # ---- PE pre-warm: dummy matmuls (result never read) so the HAM
        # clock-gate un-throttles while the first DMAs stream in ----
        warm_ps = ps.tile([P, 2 * 512], F32, tag="ps")
        for _ in range(N_WARMUP_MM):
            nc.tensor.matmul(warm_ps[:, 0:256], warm_s[:, 0:128], warm_s[:],
                             start=True, stop=True)

        for b in range(BPC):
            qt_s = qkv.tile([P, EO, S], F16, tag="qt")
            kt_s = qkv.tile([P, EO, S], F16, tag="kt")
            v_s = qkv.tile([P, KO, DA], F16, tag="v")
            # softmax row-sum ones column (col 768) + zero pad (col 769);
            # the V projection only writes cols 0:768
            nc.gpsimd.memset(v_s[:, :, D:D + 1], 1.0)
            nc.gpsimd.memset(v_s[:, :, D + 1:DA], 0.0)

            if b == 0:
                xt_t = xt0_t
            else:
                xt_t = xt_pool.tile([P, JB, EO, 256], F16, tag="xt")
                nc.scalar.dma_start(xt_t[:], xt_d[b])

            # Q/K: chunk pairs (c, c+3) into one PSUM slot, then RoPE+bias
            # out of PSUM: (q+b)*trig on DVE, combine add/sub on GPSIMD.
            def qk_group(wi, h, c):
                w_s, b_s, dst = ((wq_s, bq_s, qt_s), (wk_s, bk_s, kt_s))[wi]
                sl = slice(h * 512, (h + 1) * 512)
                slot = ps.tile([P, 2 * 512], F32, tag="ps")
                # batch-0 first Q group of each half: single-j (N=256)
                # chains, j-outer, so the PE starts on the first xt
                # j-lump before the second lands; a couple of dummy
                # matmuls bridge the lump boundary
                drip = b == 0 and wi == 0 and c == 0
                if drip:
                    for jj in range(2):
                        j = 2 * h + jj
                        for half_i in range(2):
                            o = half_i * 512 + jj * 256
                            for d in range(EO):
                                nc.tensor.matmul(
                                    slot[:, o:o + 256],
                                    w_s[:, c, d, half_i, :],
                                    xt_t[:, j, d, :],
                                    start=(d == 0), stop=(d == EO - 1),
                                )
                        if jj == 0:
                            for _ in range(4 if h == 0 else 6):
                                nc.tensor.matmul(
                                    warm_ps[:, 0:256], warm_s[:, 0:128],
                                    warm_s[:], start=True, stop=True)
                else:
                    for half_i in range(2):
                        o = half_i * 512
                        for d in range(EO):
                            nc.tensor.matmul(
                                slot[:, o:o + 512],
                                w_s[:, c, d, half_i, :],
                                xt_t[:, 2 * h:2 * h + 2, d, :],
                                start=(d == 0), stop=(d == EO - 1),
                            )
                pc = slot[:, 0:512]
                po = slot[:, 512:1024]
                cs = trig_s[:, h, 0, c, :]
                sn = trig_s[:, h, 1, c, :]
                # (psum + bias) * trig on DVE (PSUM+SBUF reads run in
                # parallel ports; SBUF+SBUF would serialize), combine
                # add/sub on GPSIMD
                # rope temps in fp16: the GPSIMD combine then reads two
                # 16-bit step-1 tensors and gets the 2x_1P packed mode
                # (one extra fp16 rounding, ~1e-4 on the output)
                ta = rope_tmp.tile([P, 512], F16, tag="rt")
                tb = rope_tmp.tile([P, 512], F16, tag="rt")
                nc.vector.scalar_tensor_tensor(
                    ta[:], pc, b_s[:, c:c + 1], cs, op0=Add, op1=Mult)
                nc.vector.scalar_tensor_tensor(
                    tb[:], po, b_s[:, c + NPAIR:c + NPAIR + 1], sn,
                    op0=Add, op1=Mult)
                nc.gpsimd.tensor_tensor(dst[:, c, sl], ta[:], tb[:], Sub)
                tc_ = rope_tmp.tile([P, 512], F16, tag="rt")
                td = rope_tmp.tile([P, 512], F16, tag="rt")
                nc.vector.scalar_tensor_tensor(
                    tc_[:], pc, b_s[:, c:c + 1], sn, op0=Add, op1=Mult)
                nc.vector.scalar_tensor_tensor(
                    td[:], po, b_s[:, c + NPAIR:c + NPAIR + 1], cs,
                    op0=Add, op1=Mult)
                nc.gpsimd.tensor_tensor(
                    dst[:, c + NPAIR, sl], tc_[:], td[:], Add)

            # V: natural layout [s, e], s-tiles of 128.  The bias bv is
            # NOT added here: it passes through the softmax average
            # unchanged (sum w (v+bv) / sum w = out + bv), so it folds
            # into the output normalize STT instead.  The drain is then
            # a pure cast, which runs on the otherwise-idle ACT engine —
            # the DVE carries the whole RoPE backlog and draining there
            # stalls the PE on PSUM slot rotation in batch 0.
            def v_group(h, st2):
                st = h * 4 + st2
                j, jh = st // 2, st % 2
                vslot = ps.tile([P, 2 * 512], F32, tag="ps")
                for off, w in ((0, 512), (512, 256)):
                    for d in range(EO):
                        nc.tensor.matmul(
                            vslot[:, off:off + w],
                            xt_t[:, j, d, jh * P:(jh + 1) * P],
                            wv_s[:, d, off:off + w],
                            start=(d == 0), stop=(d == EO - 1),
                        )
                nc.scalar.activation(v_s[:, st, 0:D], vslot[:, 0:D],
                                     Ident, scale=1.0, bias=0.0)

            if b == 0:
                # weights stream in during batch 0: Q groups first (wq +
                # xt j-lumps), then K (wk lands mid-Q), then V (wv on the
                # gpsimd SWDGE queue, issued after the first RoPE groups
                # so it doesn't steal SDMA slots from the critical path)
                for wi in range(2):
                    for h in range(2):
                        for c in range(NPAIR):
                            qk_group(wi, h, c)
                            if wi == 0 and h == 0 and c == 2:
                                # gate the wv load on a value the RoPE
                                # combine writes late, so the scheduler
                                # cannot hoist it into the startup
                                # window where it would steal SDMA
                                # round-robin slots from the critical
                                # wq/xt lumps (a bare dma_start gets
                                # reordered to t=0: gpsimd is idle then)
                                nc.gpsimd.tensor_tensor(
                                    wv_s[0:1, 0, 0:1], qt_s[0:1, 2, 0:1],
                                    qt_s[0:1, 2, 0:1], Add)
                                nc.gpsimd.dma_start(wv_s[:, 0:3, :],
                                                    wv_d[:, 0:3, :])
                            if wi == 0 and h == 1 and c == 0:
                                nc.gpsimd.tensor_tensor(
                                    wv_s[0:1, 3, 0:1], qt_s[0:1, 0, 512:513],
                                    qt_s[0:1, 0, 512:513], Add)
                                nc.gpsimd.dma_start(wv_s[:, 3:6, :],
                                                    wv_d[:, 3:6, :])
                for h in range(2):
                    for st2 in range(4):
                        v_group(h, st2)
            else:
                # interleave the DVE-free V groups between Q/K RoPE groups so
                # the DVE (3.3us/group) never falls behind the PE
                # (2.56us/group) far enough to stall PSUM slot reuse
                order = []
                for h in range(2):
                    if h == 0:
                        qks = [(0, h, 0), (0, h, 1), (0, h, 2),
                               (1, h, 0), (1, h, 1), (1, h, 2)]
                    else:
                        # K before Q in the second half: the first scores
                        # matmuls of qh0 need the K h1 k-tiles' RoPE
                        # chain (DVE+GPSIMD) drained, so K must not be
                        # the last projection group
                        qks = [(1, h, 0), (1, h, 1), (1, h, 2),
                               (0, h, 0), (0, h, 1), (0, h, 2)]
                    vs = [(h, 0), (h, 1), (h, 2), (h, 3)]
                    for i, g in enumerate(qks):
                        order.append(("qk", g))
                        if i < len(vs):
                            order.append(("v", vs[i]))
                for kind, g in order:
                    if kind == "qk":
                        qk_group(*g)
                    else:
                        v_group(*g)

            # ---- attention, in two q-halves of 512 ----
            for qh in range(2):
                expt_s = qkv.tile([P, KO, 512], F16, tag="expt")
                # scoresT[k, q-half]; two k-tiles share one PSUM slot
                for t in range(KO // 2):
                    sslot = ps.tile([P, 2 * 512], F32, tag="ps")
                    for half_i in range(2):
                        kt = 2 * t + half_i
                        for e in range(EO):
                            nc.tensor.matmul(
                                sslot[:, half_i * 512:(half_i + 1) * 512],
                                kt_s[:, e, kt * P:(kt + 1) * P],
                                qt_s[:, e, qh * 512:(qh + 1) * 512],
                                start=(e == 0), stop=(e == EO - 1),
                            )
                    nc.scalar.activation(
                        expt_s[:, 2 * t:2 * t + 2, :].rearrange("p a b -> p (a b)"),
                        sslot[:, :], Exp, scale=INV_SQRT_D)

                # out[q, e] = expT.T @ V_aug ; col 768 = softmax row sum
                tail = b == BPC - 1 and qh == 1
                for ql in range(4):
                    qt = qh * 4 + ql
                    oslot = ps.tile([P, 2 * 512], F32, tag="ps")
                    for kt in range(KO):
                        st = (kt == 0)
                        sp = (kt == KO - 1)
                        nc.tensor.matmul(
                            oslot[:, 0:512],
                            expt_s[:, kt, ql * P:(ql + 1) * P],
                            v_s[:, kt, 0:512],
                            start=st, stop=sp,
                        )
                        nc.tensor.matmul(
                            oslot[:, 512:512 + (DA - 512)],
                            expt_s[:, kt, ql * P:(ql + 1) * P],
                            v_s[:, kt, 512:DA],
                            start=st, stop=sp,
                        )
                    recip = small.tile([P, 1], F32, tag="recip")
                    nc.vector.reciprocal(recip[:], oslot[:, D:D + 1])
                    o_sb = osb_pool.tile([P, D], F32, tag="osb")
                    # normalize + V-bias in one STT: (oslot*recip) + bv.
                    # Runs on DVE, which is idle during the attention
                    # phase; ACT keeps only exp and the V-drain copies.
                    nc.vector.scalar_tensor_tensor(
                        o_sb[:, 0:512], oslot[:, 0:512], recip[:, 0:1],
                        bvb_s[:, 0:512], op0=Mult, op1=Add)
                    if tail:
                        # final q-tiles: store each half as soon as it is
                        # scaled so the first half's transfer overlaps
                        # the second half's normalize
                        nc.sync.dma_start(
                            out_d[b, qt * P:(qt + 1) * P, 0:512],
                            o_sb[:, 0:512])
                        nc.vector.scalar_tensor_tensor(
                            o_sb[:, 512:D], oslot[:, 512:D], recip[:, 0:1],
                            bvb_s[:, 512:D], op0=Mult, op1=Add)
                        nc.sync.dma_start(
                            out_d[b, qt * P:(qt + 1) * P, 512:D],
                            o_sb[:, 512:D])
                    else:
                        nc.vector.scalar_tensor_tensor(
                            o_sb[:, 512:D], oslot[:, 512:D], recip[:, 0:1],
                            bvb_s[:, 512:D], op0=Mult, op1=Add)
                        nc.sync.dma_start(
                            out_d[b, qt * P:(qt + 1) * P, :], o_sb[:, :])


def _host_prep(x, Wq, bq, Wk, bk, Wv, bv):
    perm = np.concatenate([np.arange(0, D, 2), np.arange(1, D, 2)])

    def prep_w_qk(w):
        wT = np.ascontiguousarray(w[perm].T)             # [d, e]
        # e = (pair, c, col) with chunk index = pair*NPAIR + c
        w5 = wT.reshape(EO, P, 2, NPAIR, P)              # [dchunk, dp, pair, c, col]
        return np.ascontiguousarray(
            w5.transpose(1, 3, 0, 2, 4)).astype(np.float16)  # [dp, c, dchunk, pair, col]

    def prep_wv(w):
        wT = np.ascontiguousarray(w.T)                   # [d, e]
        wT = np.concatenate([wT, np.zeros((D, 2), np.float32)], axis=1)
        return np.ascontiguousarray(
            wT.reshape(EO, P, DA).transpose(1, 0, 2)).astype(np.float16)

    wq_dev = prep_w_qk(Wq)
    wk_dev = prep_w_qk(Wk)
    wv_dev = prep_wv(Wv)
    bq_dev = bq[perm].reshape(EO, P).T
    bk_dev = bk[perm].reshape(EO, P).T
    bv_aug = np.concatenate([bv.astype(np.float32),
                             np.array([1.0, 0.0], np.float32)])
    bvb_dev = np.broadcast_to(bv_aug, (P, DA))
    bias_dev = np.ascontiguousarray(
        np.concatenate([bq_dev, bk_dev, bvb_dev], axis=1)).astype(np.float16)

    inv_freq = (1.0 / (np.float32(ROPE_BASE)
                       ** (np.arange(HALF, dtype=np.float32)
                           * np.float32(2.0) / np.float32(D)))).astype(np.float32)
    ang = np.arange(S, dtype=np.float32)[:, None] * inv_freq[None, :]  # [S, HALF]
    cosT = np.cos(ang).T.astype(np.float32)  # [HALF, S]
    sinT = np.sin(ang).T.astype(np.float32)
    # trig[dp, h, cs, c, col] = {cos,sin}T[c*P+dp, h*512+col]
    trig4 = np.stack([cosT.reshape(NPAIR, P, 2, 512),
                      sinT.reshape(NPAIR, P, 2, 512)], axis=0)  # [cs,c,dp,h,col]
    trig_dev = np.ascontiguousarray(
        trig4.transpose(2, 3, 0, 1, 4)).astype(np.float16)      # [dp,h,cs,c,col]

    xt_devs = []
    for c in range(NCORES):
        xs = x[c * BPC:(c + 1) * BPC]                # [BPC, S, D]
        xT = xs.transpose(0, 2, 1)                   # [BPC, D, S]
        x6 = xT.reshape(BPC, EO, P, JB, 256)         # [b, dchunk, dp, j, col]
        xt_devs.append(np.ascontiguousarray(
            x6.transpose(0, 2, 3, 1, 4)).astype(np.float16))  # [b, dp, j, dchunk, col]

    shared = dict(wq=wq_dev, wk=wk_dev, wv=wv_dev, bias=bias_dev,
                  trig=trig_dev)
    return [dict(xt=xt_devs[c], **shared) for c in range(NCORES)]


def kernel(x, Wq, bq, Wk, bk, Wv, bv, _trace=False):
    if "nc" not in _CACHE:
        _CACHE["nc"] = _build_nc()
    nc = _CACHE["nc"]

    in_maps = _host_prep(np.asarray(x, dtype=np.float32),
                         np.asarray(Wq, dtype=np.float32),
                         np.asarray(bq, dtype=np.float32),
                         np.asarray(Wk, dtype=np.float32),
                         np.asarray(bk, dtype=np.float32),
                         np.asarray(Wv, dtype=np.float32),
                         np.asarray(bv, dtype=np.float32))

    res = run_bass_kernel_spmd(nc, in_maps, list(range(NCORES)), trace=_trace)
    out = np.concatenate([res.results[c]["out"] for c in range(NCORES)], axis=0)
    if _trace:
        _CACHE["last_exec_time_ns"] = res.exec_time_ns
        _CACHE["last_results"] = res
    return out


# revision 32
# speedup vs baseline: 1.0078x; 1.0078x over previous
"""Single-head attention with interleaved RoPE — Trainium2 Bass kernel (v4).

Problem: B=32, S=1024, D=768 fp32.
  Q = x@Wq.T + bq; K = x@Wk.T + bk; V = x@Wv.T + bv
  Q,K := interleaved RoPE over sequence positions
  out = softmax(Q K^T / sqrt(D)) @ V

v8 (vs v3 at ~383us; the fp16 PE-streaming roofline for this layout is
348.5us and the steady state already runs exactly at roofline +2.8ns/MM
NX dispatch, so all recoverable slack is startup, batch-0, and tail):
  - fp8 was evaluated and rejected: DoubleRow doubles PE rate but exact
    numpy emulation of e4m3 quantization busts the 2e-2 gate 2.4-3.6x
    for EVERY stage (scores 5.7e-2, AV 4.8e-2), and error-compensated
    splits cost back exactly the 2x.
  - Host layouts make every batch-0 critical DMA a large-packet lump:
    wq/wk c-block-major [P,3,EO,2,128], x j-block-major [P,4,EO,256],
    trig [P,2(h),2(cos/sin),NPAIR,512], biases packed into one [P,784]
    tensor (separate bq/bk loads were 128x 24B packets, poisoning the
    SDMA round-robin).
  - The 16 SDMA engines round-robin across ACTIVE queues per packet,
    so queue order IS priority: both HW-DGE queues (sync, scalar) are
    loaded strictly by compute deadline, the first matmul group's
    operands split across both queues in parallel.  wv rides the
    gpsimd SWDGE gated behind a RoPE output read (a bare dma_start
    gets scheduler-hoisted to t=0 and steals early bandwidth).
  - 26 PE warm-up dummies (5.5us busy) guarantee the free-running
    3.4us HAM window sees a full busy window, so the clock-gate lifts
    to 2.4GHz before the first real matmuls (16 raced it and losing
    runs ran all of batch 0 at 1.2GHz).
  - RoPE temps in fp16 (GPSIMD combines get the 2x_1P packed mode);
    V-bias drains on gpsimd; K-projection before Q in the second
    half so the scores matmuls never wait on the last RoPE chain.
  - Tail: the final q-tiles store each 512/256-col half as soon as it
    is scaled so the last transfer overlaps the last activation.
"""

import numpy as np

import concourse.bass as bass
import concourse.mybir as mybir
import concourse.tile as tile
from concourse import bacc
from concourse.bass_utils import run_bass_kernel_spmd

F32 = mybir.dt.float32
F16 = mybir.dt.float16

B, S, D = 32, 1024, 768
NCORES = 8
BPC = B // NCORES          # batches per core
P = 128
EO = D // P                # 6 feature chunks
KO = S // P                # 8 sequence tiles
JB = S // 256              # 4 j-blocks of 256 seq positions
HALF = D // 2              # 384
NPAIR = HALF // P          # 3 rope chunk pairs
DA = D + 2                 # V augmented: ones column (row sums) + zero pad
ROPE_BASE = 10000.0
INV_SQRT_D = float(1.0 / np.sqrt(np.float32(D)))
N_WARMUP_MM = 26           # PE pre-warm dummy matmuls (N=256 each).
# 26 x 213ns = 5.5us of sustained PE-busy: the free-running 3.4us HAM
# window is guaranteed to see one full busy window, so the clock-gate
# lifts to 2.4GHz before the first real (DMA-gated) matmuls run.  16
# was exactly one window and raced it: runs that lost started the
# whole first batch at 1.2GHz.

_CACHE: dict = {}


def _build_nc():
    nc = bacc.Bacc("TRN2", target_bir_lowering=False, debug=False,
                   num_devices=NCORES)

    xt_d = nc.dram_tensor("xt", (BPC, P, JB, EO, 256), F16,
                          kind="ExternalInput").ap()
    wq_d = nc.dram_tensor("wq", (P, NPAIR, EO, 2, P), F16,
                          kind="ExternalInput").ap()
    wk_d = nc.dram_tensor("wk", (P, NPAIR, EO, 2, P), F16,
                          kind="ExternalInput").ap()
    wv_d = nc.dram_tensor("wv", (P, EO, DA), F16, kind="ExternalInput").ap()
    # bq | bk | bvb packed into one transfer: separate loads would each
    # be 128 tiny 24B packets, poisoning the SDMA round-robin.  fp16:
    # biases are ~0.01 magnitude, quantization ~5e-6 absolute.
    bias_d = nc.dram_tensor("bias", (P, 2 * EO + DA), F16,
                            kind="ExternalInput").ap()
    trig_d = nc.dram_tensor("trig", (P, 2, 2, NPAIR, 512), F16,
                            kind="ExternalInput").ap()
    out_d = nc.dram_tensor("out", (BPC, S, D), F32, kind="ExternalOutput").ap()

    with tile.TileContext(nc) as tc:
        _body(tc, xt_d, wq_d, wk_d, wv_d, bias_d, trig_d, out_d)
    nc.compile()
    return nc


def _body(tc, xt_d, wq_d, wk_d, wv_d, bias_d, trig_d, out_d):
    nc = tc.nc
    Add = mybir.AluOpType.add
    Sub = mybir.AluOpType.subtract
    Mult = mybir.AluOpType.mult
    Exp = mybir.ActivationFunctionType.Exp
    Ident = mybir.ActivationFunctionType.Identity

    with (
        tc.tile_pool(name="consts", bufs=1) as consts,
        tc.tile_pool(name="xt", bufs=2) as xt_pool,
        tc.tile_pool(name="qkv", bufs=1) as qkv,
        tc.tile_pool(name="rope_tmp", bufs=8) as rope_tmp,
        tc.tile_pool(name="osb", bufs=2) as osb_pool,
        tc.tile_pool(name="small", bufs=4) as small,
        tc.tile_pool(name="ps", bufs=4, space="PSUM") as ps,
    ):
        bias_s = consts.tile([P, 2 * EO + DA], F16, tag="bias")
        bq_s = bias_s[:, 0:EO]
        bk_s = bias_s[:, EO:2 * EO]
        bvb_s = bias_s[:, 2 * EO:2 * EO + DA]
        trig_s = consts.tile([P, 2, 2, NPAIR, 512], F16, tag="trig")
        wq_s = consts.tile([P, NPAIR, EO, 2, P], F16, tag="wq")
        wk_s = consts.tile([P, NPAIR, EO, 2, P], F16, tag="wk")
        wv_s = consts.tile([P, EO, DA], F16, tag="wv")

        # warm-up operand for the PE pre-warm matmuls; memset on the
        # vector engine, which clears the framework prologue early and
        # has no other work until the first RoPE group
        warm_s = consts.tile([P, 256], F16, tag="warm")
        nc.vector.memset(warm_s[:], 0.0)

        # batch-0 critical-path loads.  sync and scalar are the two
        # HW-DGE queues; the 16 SDMA engines round-robin across active
        # queues per packet, so order IS priority.  First real matmul
        # group needs wq-c0 (sync lump 1) + xt-j0 (scalar lump 1).
        # batch-0 loads, strictly ordered by compute deadline on each of
        # the two HW-DGE queues (the 16 SDMA engines round-robin across
        # active queues per packet; aggregate ~0.36 MB/us with both
        # active).  The first matmul group's operands (wq-c0, xt-j0) are
        # split across BOTH queues so each rides half the bandwidth in
        # parallel.  DVE-side inputs (trig, bias) may land ~2us past
        # their first use — the PSUM slot rotation absorbs that; PE-side
        # inputs (wq/wk/xt lumps) must not be late.
        xt0_t = xt_pool.tile([P, JB, EO, 256], F16, tag="xt")
        nc.sync.dma_start(wq_s[:, 0, 0:3], wq_d[:, 0, 0:3])
        nc.scalar.dma_start(wq_s[:, 0, 3:6], wq_d[:, 0, 3:6])
        nc.sync.dma_start(xt0_t[:, 0, 0:3], xt_d[0, :, 0, 0:3])
        nc.scalar.dma_start(xt0_t[:, 0, 3:6], xt_d[0, :, 0, 3:6])
        nc.sync.dma_start(wq_s[:, 1], wq_d[:, 1])
        nc.scalar.dma_start(xt0_t[:, 1], xt_d[0, :, 1])
        nc.sync.dma_start(trig_s[:, 0, 0, 0], trig_d[:, 0, 0, 0])
        nc.scalar.dma_start(bias_s[:], bias_d[:])
        nc.sync.dma_start(wq_s[:, 2], wq_d[:, 2])
        nc.scalar.dma_start(trig_s[:, 0, 1, 0], trig_d[:, 0, 1, 0])
        nc.sync.dma_start(wk_s[:, 0], wk_d[:, 0])
        nc.scalar.dma_start(xt0_t[:, 2], xt_d[0, :, 2])
        nc.sync.dma_start(trig_s[:, 0, 0, 1], trig_d[:, 0, 0, 1])
        nc.scalar.dma_start(trig_s[:, 0, 1, 1], trig_d[:, 0, 1, 1])
        nc.sync.dma_start(trig_s[:, 0, 0, 2], trig_d[:, 0, 0, 2])
        nc.scalar.dma_start(trig_s[:, 0, 1, 2], trig_d[:, 0, 1, 2])
        nc.scalar.dma_start(xt0_t[:, 3], xt_d[0, :, 3])
        nc.sync.dma_start(trig_s[:, 1, 0, 0], trig_d[:, 1, 0, 0])
        nc.scalar.dma_start(trig_s[:, 1, 1, 0], trig_d[:, 1, 1, 0])
        nc.sync.dma_start(trig_s[:, 1, 0, 1], trig_d[:, 1, 0, 1])
        nc.scalar.dma_start(trig_s[:, 1, 1, 1], trig_d[:, 1, 1, 1])
        nc.sync.dma_start(trig_s[:, 1, 0, 2], trig_d[:, 1, 0, 2])
        nc.scalar.dma_start(trig_s[:, 1, 1, 2], trig_d[:, 1, 1, 2])
        nc.sync.dma_start(wk_s[:, 1], wk_d[:, 1])
        nc.sync.dma_start(wk_s[:, 2], wk_d[:, 2])

        # ---- PE pre-warm: dummy matmuls (result never read) so the HAM
        # clock-gate un-throttles while the first DMAs stream in ----
        warm_ps = ps.tile([P, 2 * 512], F32, tag="ps")
        for _ in range(N_WARMUP_MM):
            nc.tensor.matmul(warm_ps[:, 0:256], warm_s[:, 0:128], warm_s[:],
                             start=True, stop=True)

        for b in range(BPC):
            qt_s = qkv.tile([P, EO, S], F16, tag="qt")
            kt_s = qkv.tile([P, EO, S], F16, tag="kt")
            v_s = qkv.tile([P, KO, DA], F16, tag="v")

            if b == 0:
                xt_t = xt0_t
            else:
                xt_t = xt_pool.tile([P, JB, EO, 256], F16, tag="xt")
                nc.scalar.dma_start(xt_t[:], xt_d[b])

            # Q/K: chunk pairs (c, c+3) into one PSUM slot, then RoPE+bias
            # out of PSUM: (q+b)*trig on DVE, combine add/sub on GPSIMD.
            def qk_group(wi, h, c):
                w_s, b_s, dst = ((wq_s, bq_s, qt_s), (wk_s, bk_s, kt_s))[wi]
                sl = slice(h * 512, (h + 1) * 512)
                slot = ps.tile([P, 2 * 512], F32, tag="ps")
                # batch-0 first Q group of each half: single-j (N=256)
                # chains, j-outer, so the PE starts on the first xt
                # j-lump before the second lands; a couple of dummy
                # matmuls bridge the lump boundary
                drip = b == 0 and wi == 0 and c == 0
                if drip:
                    for jj in range(2):
                        j = 2 * h + jj
                        for half_i in range(2):
                            o = half_i * 512 + jj * 256
                            for d in range(EO):
                                nc.tensor.matmul(
                                    slot[:, o:o + 256],
                                    w_s[:, c, d, half_i, :],
                                    xt_t[:, j, d, :],
                                    start=(d == 0), stop=(d == EO - 1),
                                )
                        if jj == 0:
                            for _ in range(4 if h == 0 else 6):
                                nc.tensor.matmul(
                                    warm_ps[:, 0:256], warm_s[:, 0:128],
                                    warm_s[:], start=True, stop=True)
                else:
                    for half_i in range(2):
                        o = half_i * 512
                        for d in range(EO):
                            nc.tensor.matmul(
                                slot[:, o:o + 512],
                                w_s[:, c, d, half_i, :],
                                xt_t[:, 2 * h:2 * h + 2, d, :],
                                start=(d == 0), stop=(d == EO - 1),
                            )
                pc = slot[:, 0:512]
                po = slot[:, 512:1024]
                cs = trig_s[:, h, 0, c, :]
                sn = trig_s[:, h, 1, c, :]
                # (psum + bias) * trig on DVE (PSUM+SBUF reads run in
                # parallel ports; SBUF+SBUF would serialize), combine
                # add/sub on GPSIMD
                # rope temps in fp16: the GPSIMD combine then reads two
                # 16-bit step-1 tensors and gets the 2x_1P packed mode
                # (one extra fp16 rounding, ~1e-4 on the output)
                ta = rope_tmp.tile([P, 512], F16, tag="rt")
                tb = rope_tmp.tile([P, 512], F16, tag="rt")
                nc.vector.scalar_tensor_tensor(
                    ta[:], pc, b_s[:, c:c + 1], cs, op0=Add, op1=Mult)
                nc.vector.scalar_tensor_tensor(
                    tb[:], po, b_s[:, c + NPAIR:c + NPAIR + 1], sn,
                    op0=Add, op1=Mult)
                nc.gpsimd.tensor_tensor(dst[:, c, sl], ta[:], tb[:], Sub)
                tc_ = rope_tmp.tile([P, 512], F16, tag="rt")
                td = rope_tmp.tile([P, 512], F16, tag="rt")
                nc.vector.scalar_tensor_tensor(
                    tc_[:], pc, b_s[:, c:c + 1], sn, op0=Add, op1=Mult)
                nc.vector.scalar_tensor_tensor(
                    td[:], po, b_s[:, c + NPAIR:c + NPAIR + 1], cs,
                    op0=Add, op1=Mult)
                nc.gpsimd.tensor_tensor(
                    dst[:, c + NPAIR, sl], tc_[:], td[:], Add)

            # V: natural layout [s, e+2], s-tiles of 128
            def v_group(h, st2):
                st = h * 4 + st2
                j, jh = st // 2, st % 2
                vslot = ps.tile([P, 2 * 512], F32, tag="ps")
                for off, w in ((0, 512), (512, DA - 512)):
                    for d in range(EO):
                        nc.tensor.matmul(
                            vslot[:, off:off + w],
                            xt_t[:, j, d, jh * P:(jh + 1) * P],
                            wv_s[:, d, off:off + w],
                            start=(d == 0), stop=(d == EO - 1),
                        )
                # must stay on DVE: GPSIMD has no PSUM port, and ACT's
                # strict-FIFO queue would serialize these ahead of exp
                nc.vector.tensor_tensor(
                    v_s[:, st, :], vslot[:, 0:DA], bvb_s[:], Add)

            if b == 0:
                # weights stream in during batch 0: Q groups first (wq +
                # xt j-lumps), then K (wk lands mid-Q), then V (wv on the
                # gpsimd SWDGE queue, issued after the first RoPE groups
                # so it doesn't steal SDMA slots from the critical path)
                for wi in range(2):
                    for h in range(2):
                        for c in range(NPAIR):
                            qk_group(wi, h, c)
                            if wi == 0 and h == 0 and c == 2:
                                # gate the wv load on a value the RoPE
                                # combine writes late, so the scheduler
                                # cannot hoist it into the startup
                                # window where it would steal SDMA
                                # round-robin slots from the critical
                                # wq/xt lumps (a bare dma_start gets
                                # reordered to t=0: gpsimd is idle then)
                                nc.gpsimd.tensor_tensor(
                                    wv_s[0:1, 0, 0:1], qt_s[0:1, 2, 0:1],
                                    qt_s[0:1, 2, 0:1], Add)
                                nc.gpsimd.dma_start(wv_s[:, 0:3, :],
                                                    wv_d[:, 0:3, :])
                            if wi == 0 and h == 1 and c == 0:
                                nc.gpsimd.tensor_tensor(
                                    wv_s[0:1, 3, 0:1], qt_s[0:1, 0, 512:513],
                                    qt_s[0:1, 0, 512:513], Add)
                                nc.gpsimd.dma_start(wv_s[:, 3:6, :],
                                                    wv_d[:, 3:6, :])
                for h in range(2):
                    for st2 in range(4):
                        v_group(h, st2)
            else:
                # interleave the DVE-free V groups between Q/K RoPE groups so
                # the DVE (3.3us/group) never falls behind the PE
                # (2.56us/group) far enough to stall PSUM slot reuse
                order = []
                for h in range(2):
                    if h == 0:
                        qks = [(0, h, 0), (0, h, 1), (0, h, 2),
                               (1, h, 0), (1, h, 1), (1, h, 2)]
                    else:
                        # K before Q in the second half: the first scores
                        # matmuls of qh0 need the K h1 k-tiles' RoPE
                        # chain (DVE+GPSIMD) drained, so K must not be
                        # the last projection group
                        qks = [(1, h, 0), (1, h, 1), (1, h, 2),
                               (0, h, 0), (0, h, 1), (0, h, 2)]
                    vs = [(h, 0), (h, 1), (h, 2), (h, 3)]
                    for i, g in enumerate(qks):
                        order.append(("qk", g))
                        if i < len(vs):
                            order.append(("v", vs[i]))
                for kind, g in order:
                    if kind == "qk":
                        qk_group(*g)
                    else:
                        v_group(*g)

            # ---- attention, in two q-halves of 512 ----
            for qh in range(2):
                expt_s = qkv.tile([P, KO, 512], F16, tag="expt")
                # scoresT[k, q-half]; two k-tiles share one PSUM slot
                for t in range(KO // 2):
                    sslot = ps.tile([P, 2 * 512], F32, tag="ps")
                    for half_i in range(2):
                        kt = 2 * t + half_i
                        for e in range(EO):
                            nc.tensor.matmul(
                                sslot[:, half_i * 512:(half_i + 1) * 512],
                                kt_s[:, e, kt * P:(kt + 1) * P],
                                qt_s[:, e, qh * 512:(qh + 1) * 512],
                                start=(e == 0), stop=(e == EO - 1),
                            )
                    nc.scalar.activation(
                        expt_s[:, 2 * t:2 * t + 2, :].rearrange("p a b -> p (a b)"),
                        sslot[:, :], Exp, scale=INV_SQRT_D)

                # out[q, e] = expT.T @ V_aug ; col 768 = softmax row sum
                tail = b == BPC - 1 and qh == 1
                for ql in range(4):
                    qt = qh * 4 + ql
                    oslot = ps.tile([P, 2 * 512], F32, tag="ps")
                    for kt in range(KO):
                        st = (kt == 0)
                        sp = (kt == KO - 1)
                        nc.tensor.matmul(
                            oslot[:, 0:512],
                            expt_s[:, kt, ql * P:(ql + 1) * P],
                            v_s[:, kt, 0:512],
                            start=st, stop=sp,
                        )
                        nc.tensor.matmul(
                            oslot[:, 512:512 + (DA - 512)],
                            expt_s[:, kt, ql * P:(ql + 1) * P],
                            v_s[:, kt, 512:DA],
                            start=st, stop=sp,
                        )
                    recip = small.tile([P, 1], F32, tag="recip")
                    nc.vector.reciprocal(recip[:], oslot[:, D:D + 1])
                    o_sb = osb_pool.tile([P, D], F32, tag="osb")
                    nc.scalar.activation(o_sb[:, 0:512], oslot[:, 0:512],
                                         Ident, scale=recip[:, 0:1],
                                         bias=0.0)
                    if tail:
                        # final q-tiles: store each half as soon as it is
                        # scaled so the first half's transfer overlaps
                        # the second half's activation
                        nc.sync.dma_start(
                            out_d[b, qt * P:(qt + 1) * P, 0:512],
                            o_sb[:, 0:512])
                        nc.scalar.activation(o_sb[:, 512:D], oslot[:, 512:D],
                                             Ident, scale=recip[:, 0:1],
                                             bias=0.0)
                        nc.sync.dma_start(
                            out_d[b, qt * P:(qt + 1) * P, 512:D],
                            o_sb[:, 512:D])
                    else:
                        nc.scalar.activation(o_sb[:, 512:D], oslot[:, 512:D],
                                             Ident, scale=recip[:, 0:1],
                                             bias=0.0)
                        nc.sync.dma_start(
                            out_d[b, qt * P:(qt + 1) * P, :], o_sb[:, :])


def _host_prep(x, Wq, bq, Wk, bk, Wv, bv):
    perm = np.concatenate([np.arange(0, D, 2), np.arange(1, D, 2)])

    def prep_w_qk(w):
        wT = np.ascontiguousarray(w[perm].T)             # [d, e]
        # e = (pair, c, col) with chunk index = pair*NPAIR + c
        w5 = wT.reshape(EO, P, 2, NPAIR, P)              # [dchunk, dp, pair, c, col]
        return np.ascontiguousarray(
            w5.transpose(1, 3, 0, 2, 4)).astype(np.float16)  # [dp, c, dchunk, pair, col]

    def prep_wv(w):
        wT = np.ascontiguousarray(w.T)                   # [d, e]
        wT = np.concatenate([wT, np.zeros((D, 2), np.float32)], axis=1)
        return np.ascontiguousarray(
            wT.reshape(EO, P, DA).transpose(1, 0, 2)).astype(np.float16)

    wq_dev = prep_w_qk(Wq)
    wk_dev = prep_w_qk(Wk)
    wv_dev = prep_wv(Wv)
    bq_dev = bq[perm].reshape(EO, P).T
    bk_dev = bk[perm].reshape(EO, P).T
    bv_aug = np.concatenate([bv.astype(np.float32),
                             np.array([1.0, 0.0], np.float32)])
    bvb_dev = np.broadcast_to(bv_aug, (P, DA))
    bias_dev = np.ascontiguousarray(
        np.concatenate([bq_dev, bk_dev, bvb_dev], axis=1)).astype(np.float16)

    inv_freq = (1.0 / (np.float32(ROPE_BASE)
                       ** (np.arange(HALF, dtype=np.float32)
                           * np.float32(2.0) / np.float32(D)))).astype(np.float32)
    ang = np.arange(S, dtype=np.float32)[:, None] * inv_freq[None, :]  # [S, HALF]
    cosT = np.cos(ang).T.astype(np.float32)  # [HALF, S]
    sinT = np.sin(ang).T.astype(np.float32)
    # trig[dp, h, cs, c, col] = {cos,sin}T[c*P+dp, h*512+col]
    trig4 = np.stack([cosT.reshape(NPAIR, P, 2, 512),
                      sinT.reshape(NPAIR, P, 2, 512)], axis=0)  # [cs,c,dp,h,col]
    trig_dev = np.ascontiguousarray(
        trig4.transpose(2, 3, 0, 1, 4)).astype(np.float16)      # [dp,h,cs,c,col]

    xt_devs = []
    for c in range(NCORES):
        xs = x[c * BPC:(c + 1) * BPC]                # [BPC, S, D]
        xT = xs.transpose(0, 2, 1)                   # [BPC, D, S]
        x6 = xT.reshape(BPC, EO, P, JB, 256)         # [b, dchunk, dp, j, col]
        xt_devs.append(np.ascontiguousarray(
            x6.transpose(0, 2, 3, 1, 4)).astype(np.float16))  # [b, dp, j, dchunk, col]

    shared = dict(wq=wq_dev, wk=wk_dev, wv=wv_dev, bias=bias_dev,
                  trig=trig_dev)
    return [dict(xt=xt_devs[c], **shared) for c in range(NCORES)]


def kernel(x, Wq, bq, Wk, bk, Wv, bv, _trace=False):
    if "nc" not in _CACHE:
        _CACHE["nc"] = _build_nc()
    nc = _CACHE["nc"]

    in_maps = _host_prep(np.asarray(x, dtype=np.float32),
                         np.asarray(Wq, dtype=np.float32),
                         np.asarray(bq, dtype=np.float32),
                         np.asarray(Wk, dtype=np.float32),
                         np.asarray(bk, dtype=np.float32),
                         np.asarray(Wv, dtype=np.float32),
                         np.asarray(bv, dtype=np.float32))

    res = run_bass_kernel_spmd(nc, in_maps, list(range(NCORES)), trace=_trace)
    out = np.concatenate([res.results[c]["out"] for c in range(NCORES)], axis=0)
    if _trace:
        _CACHE["last_exec_time_ns"] = res.exec_time_ns
        _CACHE["last_results"] = res
    return out
